# revision 8
# baseline (speedup 1.0000x reference)
"""RNN-T JointNetwork kernel for 8 Trainium2 NeuronCores (raw bass).

reference:
  e = enc @ W_enc.T + b_enc          # [B,T,H]
  d = dec @ W_dec.T + b_dec          # [B,U,H]
  j = tanh(e[:,:,None,:] + d[:,None,:,:])
  out = j @ W_joint.T + b_joint      # [B,T,U,V]

Sharding: T (256) split 8 ways -> 32 t-rows per core; host concatenates
along T.

Per-core dataflow (H on partitions for everything left of the big matmul):
  PE:  E^T[h, m] and D^T[h, n] projections, then per (b,t) row m the vocab
       matmul psum[u, v] += Jt[h, u]^T W_joint^T[h, v] (4 k-tiles x 2 v-banks)
  ACT: Jt[h, u] = tanh(D^T[h, (b,u)] + E^T[h, m]) via per-partition bias port
  DVE: drains psum -> sbuf while adding replicated b_joint
  SP:  all DMA (inputs once, one 512KB contiguous output row per m)

This toolchain's walrus rejects any compute instruction carrying >=2 sync
waits, so the kernel is written in raw bass: all cross-engine waits are
standalone wait_ge instructions and compute instructions carry none.
"""

import numpy as np

B, T, U = 4, 256, 128
ENC_DIM = DEC_DIM = HID = 512
VOCAB = 1024
NCORES = 8
TC = T // NCORES        # 32 t-rows per core
M = B * TC              # 128 (b,t) rows per core
KT = HID // 128         # 4 contraction tiles
HT = HID // 128         # 4 h tiles
NJT = 4                 # jt double-buffers
NOT = 6                 # output staging buffers
NPS = 4                 # psum tiles in flight (8 banks total)

_CACHE = {}


def _build_bass(reps=1, timing=False):
    import concourse.bass as bass
    import concourse.mybir as mybir

    f32 = mybir.dt.float32
    f32r = mybir.dt.float32r
    Tanh = mybir.ActivationFunctionType.Tanh

    nc = bass.Bass()
    encT = nc.declare_dram_parameter("encT", [128, KT, M], f32, isOutput=False)
    decT = nc.declare_dram_parameter("decT", [128, KT, B * U], f32, isOutput=False)
    WencT = nc.declare_dram_parameter("WencT", [128, KT, HID], f32, isOutput=False)
    WdecT = nc.declare_dram_parameter("WdecT", [128, KT, HID], f32, isOutput=False)
    WjT = nc.declare_dram_parameter("WjT", [128, HT, VOCAB], f32, isOutput=False)
    bsum = nc.declare_dram_parameter("bsum", [128, HT], f32, isOutput=False)
    bj = nc.declare_dram_parameter("bj", [128, VOCAB], f32, isOutput=False)
    if timing:
        out = nc.dram_tensor("out_i", [M, U, VOCAB], f32)
        tok = nc.declare_dram_parameter("tok", [128, 4], f32, isOutput=True)
    else:
        out = nc.declare_dram_parameter("out", [M, U, VOCAB], f32, isOutput=True)

    N_IN_DMA = 7

    from contextlib import ExitStack

    with ExitStack() as ctx:
        e = ctx.enter_context
        encT_sb = e(nc.sbuf_tensor("encT_sb", [128, KT, M], f32))
        decT_sb = e(nc.sbuf_tensor("decT_sb", [128, KT, B * U], f32))
        WencT_sb = e(nc.sbuf_tensor("WencT_sb", [128, KT, HID], f32))
        WdecT_sb = e(nc.sbuf_tensor("WdecT_sb", [128, KT, HID], f32))
        WjT_stage = e(nc.sbuf_tensor("WjT_stage", [128, HT, VOCAB], f32))
        WjT_sb = e(nc.sbuf_tensor("WjT_sb", [128, HT, VOCAB], f32r))
        bsum_sb = e(nc.sbuf_tensor("bsum_sb", [128, HT], f32))
        bj_sb = e(nc.sbuf_tensor("bj_sb", [128, VOCAB], f32))
        ET_sb = e(nc.sbuf_tensor("ET_sb", [128, HT, M], f32))
        DT_sb = e(nc.sbuf_tensor("DT_sb", [128, HT, B * U], f32))
        # ACT writes f32 (f32r ACT output is ~4x slower per instruction);
        # DVE re-rounds each tile to f32r for the full-rate PE matmul.
        jt_sb = e(nc.sbuf_tensor("jt_sb", [128, NJT, HT, 128], f32))
        jt_r = e(nc.sbuf_tensor("jt_r", [128, NJT, HT, 128], f32r))
        ot_sb = e(nc.sbuf_tensor("ot_sb", [128, NOT, VOCAB], f32))
        ps = [
            e(nc.psum_tensor(f"ps{i}", [128, VOCAB], f32)) for i in range(NPS)
        ]
        s_in = e(nc.semaphore("s_in"))
        s_act = e(nc.semaphore("s_act"))
        s_cp = e(nc.semaphore("s_cp"))
        s_pe = e(nc.semaphore("s_pe"))
        s_dve = e(nc.semaphore("s_dve"))
        s_outd = e(nc.semaphore("s_outd"))
        block = e(nc.Block())

        @block.sync
        def _(sync):
            for sb, dr in (
                (encT_sb, encT),
                (decT_sb, decT),
                (WencT_sb, WencT),
                (WdecT_sb, WdecT),
                (WjT_stage, WjT),
                (bsum_sb, bsum),
                (bj_sb, bj),
            ):
                sync.dma_start(out=sb[:], in_=dr[:]).then_inc(s_in, 16)
            for rep in range(reps):
                for m in range(M):
                    c = rep * M + m
                    sync.wait_ge(s_dve, 9 + c + 1)
                    sync.dma_start(out=out[m], in_=ot_sb[:, c % NOT, :]).then_inc(
                        s_outd, 16
                    )
            sync.wait_ge(s_outd, 16 * M * reps)
            if timing:
                sync.dma_start(out=tok[:], in_=bsum_sb[:]).then_inc(s_in, 16)
                sync.wait_ge(s_in, 16 * (N_IN_DMA + 1))

        @block.tensor
        def _(pe):
            pe.wait_ge(s_in, 16 * N_IN_DMA)
            # E^T: ps[hi][:, 0:M] (bank 2*hi)
            for hi in range(HT):
                for ki in range(KT):
                    mm = pe.matmul(
                        ps[hi][:, 0:M],
                        WencT_sb[:, ki, hi * 128 : (hi + 1) * 128],
                        encT_sb[:, ki, :],
                        start=(ki == 0),
                        stop=(ki == KT - 1),
                    )
                mm.then_inc(s_pe, 1)
            # D^T: ps[hi][:, 512:1024] (bank 2*hi+1)
            for hi in range(HT):
                for ki in range(KT):
                    mm = pe.matmul(
                        ps[hi][:, 512 : 512 + B * U],
                        WdecT_sb[:, ki, hi * 128 : (hi + 1) * 128],
                        decT_sb[:, ki, :],
                        start=(ki == 0),
                        stop=(ki == KT - 1),
                    )
                mm.then_inc(s_pe, 1)
            # main loop: s_pe = 8 + c + 1 after group c.  The single s_cp
            # wait also implies: setup drains done (DVE emits copy 0 after
            # setup), and psum slot c%NPS drained (DVE drains c-1 before
            # emitting copy c, and NPS > 1).
            for rep in range(reps):
                for m in range(M):
                    c = rep * M + m
                    pe.wait_ge(s_cp, c + 1)
                    for hi in range(HT):
                        for vi in range(2):
                            mm = pe.matmul(
                                ps[c % NPS][:, vi * 512 : (vi + 1) * 512],
                                jt_r[:, c % NJT, hi, :],
                                WjT_sb[:, hi, vi * 512 : (vi + 1) * 512],
                                start=(hi == 0),
                                stop=(hi == HT - 1),
                            )
                    mm.then_inc(s_pe, 1)

        @block.scalar
        def _(act):
            act.wait_ge(s_dve, 8)  # ET/DT ready
            for rep in range(reps):
                for m in range(M):
                    c = rep * M + m
                    b = m // TC
                    if c >= NJT:
                        act.wait_ge(s_cp, (c - NJT) + 1)  # jt_sb slot free
                    for hi in range(HT):
                        a = act.activation(
                            jt_sb[:, c % NJT, hi, :],
                            DT_sb[:, hi, b * 128 : (b + 1) * 128],
                            Tanh,
                            bias=ET_sb[:, hi, m : m + 1],
                        )
                    a.then_inc(s_act, 1)

        @block.vector
        def _(dve):
            dve.wait_ge(s_in, 16 * N_IN_DMA)
            for hi in range(HT):
                dve.wait_ge(s_pe, hi + 1)
                dve.tensor_copy(ET_sb[:, hi, :], ps[hi][:, 0:M]).then_inc(s_dve, 1)
            for hi in range(HT):
                dve.wait_ge(s_pe, 4 + hi + 1)
                dve.tensor_scalar_add(
                    DT_sb[:, hi, :],
                    ps[hi][:, 512 : 512 + B * U],
                    bsum_sb[:, hi : hi + 1],
                ).then_inc(s_dve, 1)
            dve.tensor_copy(WjT_sb[:], WjT_stage[:]).then_inc(s_dve, 1)
            # s_dve = 9 after setup.  Main loop: f32->f32r re-round of jt
            # (s_cp), then psum drain of the previous m (s_dve), keeping PE
            # a single s_cp wait per m.
            last = reps * M - 1

            def drain(c):
                dve.wait_ge(s_pe, 8 + c + 1)
                if c >= NOT:
                    dve.wait_ge(s_outd, 16 * ((c - NOT) + 1))  # ot slot free
                dve.tensor_tensor(
                    ot_sb[:, c % NOT, :],
                    ps[c % NPS][:, :],
                    bj_sb[:, :],
                    mybir.AluOpType.add,
                ).then_inc(s_dve, 1)

            for rep in range(reps):
                for m in range(M):
                    c = rep * M + m
                    dve.wait_ge(s_act, c + 1)
                    if c >= NJT:
                        dve.wait_ge(s_pe, 8 + (c - NJT) + 1)  # jt_r slot free
                    dve.tensor_copy(
                        jt_r[:, c % NJT, :, :], jt_sb[:, c % NJT, :, :]
                    ).then_inc(s_cp, 1)
                    if c >= 1:
                        drain(c - 1)
            drain(last)

    return nc


def _tile_k(a):
    """[K, X] -> [128, K//128, X] with k = kt*128 + p."""
    k, x = a.shape
    return np.ascontiguousarray(a.reshape(k // 128, 128, x).transpose(1, 0, 2))


def _prep_inputs(enc_out, dec_out, W_enc, b_enc, W_dec, b_dec, W_joint, b_joint):
    enc_out = np.asarray(enc_out, dtype=np.float32)
    dec_out = np.asarray(dec_out, dtype=np.float32)
    common = {
        "decT": _tile_k(np.ascontiguousarray(dec_out.reshape(B * U, DEC_DIM).T)),
        "WencT": _tile_k(np.ascontiguousarray(np.asarray(W_enc, np.float32).T)),
        "WdecT": _tile_k(np.ascontiguousarray(np.asarray(W_dec, np.float32).T)),
        "WjT": _tile_k(np.ascontiguousarray(np.asarray(W_joint, np.float32).T)),
        "bsum": np.ascontiguousarray(
            (np.asarray(b_enc, np.float32) + np.asarray(b_dec, np.float32))
            .reshape(HT, 128)
            .T
        ),
        "bj": np.ascontiguousarray(
            np.broadcast_to(np.asarray(b_joint, np.float32), (128, VOCAB))
        ),
    }
    in_maps = []
    for i in range(NCORES):
        sl = enc_out[:, i * TC : (i + 1) * TC, :].reshape(M, ENC_DIM)
        m = dict(common)
        m["encT"] = _tile_k(np.ascontiguousarray(sl.T))
        in_maps.append(m)
    return in_maps


def run(in_maps, trace=False, **kw):
    from concourse.bass_utils import run_bass_kernel_spmd

    if "nc" not in _CACHE:
        _CACHE["nc"] = _build_bass()
    return run_bass_kernel_spmd(
        _CACHE["nc"], in_maps, list(range(NCORES)), trace=trace, **kw
    )


def time_kernel(in_maps, reps_list=(1, 9), n_meas=3):
    """HW time per main-loop pass via rep-count wall-clock deltas.

    Timing variants write to internal DRAM (tiny external output), so the
    axon transfer cost is identical across rep counts and cancels in the
    delta.
    """
    import time
    from concourse.bass_utils import run_bass_kernel_spmd

    walls = {}
    for reps in reps_list:
        key = f"t{reps}"
        if key not in _CACHE:
            _CACHE[key] = _build_bass(reps=reps, timing=True)
        nc = _CACHE[key]
        run_bass_kernel_spmd(nc, in_maps, list(range(NCORES)))  # compile+warm
        ts = []
        for _ in range(n_meas):
            t0 = time.time()
            run_bass_kernel_spmd(nc, in_maps, list(range(NCORES)))
            ts.append(time.time() - t0)
        walls[reps] = min(ts)
    r0, r1 = reps_list
    per_pass = (walls[r1] - walls[r0]) / (r1 - r0)
    return per_pass, walls


def kernel(enc_out, dec_out, W_enc, b_enc, W_dec, b_dec, W_joint, b_joint):
    import sys

    if "/opt/trn_rl_repo" not in sys.path:
        sys.path.insert(0, "/opt/trn_rl_repo")

    in_maps = _prep_inputs(
        enc_out, dec_out, W_enc, b_enc, W_dec, b_dec, W_joint, b_joint
    )
    res = run(in_maps)
    parts = [r["out"].reshape(B, TC, U, VOCAB) for r in res.results]
    return np.concatenate(parts, axis=1)



# revision 16
# speedup vs baseline: 1.4089x; 1.4089x over previous
"""RNN-T JointNetwork kernel for 8 Trainium2 NeuronCores (raw bass).

reference:
  e = enc @ W_enc.T + b_enc          # [B,T,H]
  d = dec @ W_dec.T + b_dec          # [B,U,H]
  j = tanh(e[:,:,None,:] + d[:,None,:,:])
  out = j @ W_joint.T + b_joint      # [B,T,U,V]

Sharding: T (256) split 8 ways -> 32 t-rows per core; host concatenates
along T.

This platform charges a large, roughly flat cost per engine instruction
and a multi-ms latency for cross-engine dependencies that actually
block.  The kernel therefore (a) minimizes instruction count per
engine, (b) avoids ACT entirely (its activation instructions are 2-10x
more expensive than DVE's tensor ops here), and (c) has NO blocking
cycles between engines: DVE is the single producer (sums + deg-6
polynomial tanh -> f32r), PE consumes with one wait per m-pair, drains
are emitted by DVE *before* each pair so PE's psum slots are implied
free, and SP just streams the output DMAs.

Per-core dataflow:
  PE:   E^T[h,m], D^T[h,n] projections (f32, setup); per m-pair the
        vocab matmul psum[u,v] += jt[h,u]^T WjT^T[h,v] (f32r, 4 k-tiles
        x 2 v-banks per m)
  DVE:  setup drains (ET/DT + bias fold, WjT f32->f32r cast); per pair:
        2 psum drains (+b_joint), 8 fused sum+clamp tensor_scalar ops,
        13-op polynomial tanh -> jt_r (f32r)
  SP:   all DMA (inputs once, one 512KB contiguous output row per m)

tanh(x) ~ xc*P(xc^2), xc = clamp(x, +-3.75), P minimax deg-6 in x^2
(max abs err 2.7e-3, well under the 2e-2 gate).
"""

import numpy as np

B, T, U = 4, 256, 128
ENC_DIM = DEC_DIM = HID = 512
VOCAB = 1024
NCORES = 8
TC = T // NCORES        # 32 t-rows per core
M = B * TC              # 128 (b,t) rows per core
KT = HID // 128         # 4 contraction tiles
HT = HID // 128         # 4 h tiles
NJT = 4                 # jt_r pair-ring depth
NOT = 6                 # output staging buffers
NPS = 4                 # psum tiles in flight

# tanh(x) ~ xc*(C[0] + C[1] y + ... + C[6] y^6), y = xc^2, xc clamped
CLAMP = 3.75
POLY_C = [
    9.868656054e-01,
    -2.815523407e-01,
    6.868982108e-02,
    -1.060985507e-02,
    9.525619066e-04,
    -4.506074475e-05,
    8.650786272e-07,
]

_CACHE = {}


def _build_bass(reps=1, timing=False):
    import concourse.bass as bass
    import concourse.mybir as mybir

    f32 = mybir.dt.float32
    f32r = mybir.dt.float32r
    Add = mybir.AluOpType.add
    Mult = mybir.AluOpType.mult
    Max = mybir.AluOpType.max
    Min = mybir.AluOpType.min

    nc = bass.Bass()
    encT = nc.declare_dram_parameter("encT", [128, KT, M], f32, isOutput=False)
    decT = nc.declare_dram_parameter("decT", [128, KT, B * U], f32, isOutput=False)
    WencT = nc.declare_dram_parameter("WencT", [128, KT, HID], f32, isOutput=False)
    WdecT = nc.declare_dram_parameter("WdecT", [128, KT, HID], f32, isOutput=False)
    WjT = nc.declare_dram_parameter("WjT", [128, HT, VOCAB], f32, isOutput=False)
    bsum = nc.declare_dram_parameter("bsum", [128, HT], f32, isOutput=False)
    bj = nc.declare_dram_parameter("bj", [128, VOCAB], f32, isOutput=False)
    pc = nc.declare_dram_parameter("pc", [128, 12], f32, isOutput=False)
    if timing:
        out = nc.dram_tensor("out_i", [M, U, VOCAB], f32)
        tok = nc.declare_dram_parameter("tok", [128, 4], f32, isOutput=True)
    else:
        out = nc.declare_dram_parameter("out", [M, U, VOCAB], f32, isOutput=True)

    N_IN_DMA = 8
    NPAIR = M // 2  # 64 pairs per pass

    from contextlib import ExitStack

    with ExitStack() as ctx:
        e = ctx.enter_context
        encT_sb = e(nc.sbuf_tensor("encT_sb", [128, KT, M], f32))
        decT_sb = e(nc.sbuf_tensor("decT_sb", [128, KT, B * U], f32))
        WencT_sb = e(nc.sbuf_tensor("WencT_sb", [128, KT, HID], f32))
        WdecT_sb = e(nc.sbuf_tensor("WdecT_sb", [128, KT, HID], f32))
        WjT_stage = e(nc.sbuf_tensor("WjT_stage", [128, HT, VOCAB], f32))
        WjT_sb = e(nc.sbuf_tensor("WjT_sb", [128, HT, VOCAB], f32r))
        bsum_sb = e(nc.sbuf_tensor("bsum_sb", [128, HT], f32))
        bj_sb = e(nc.sbuf_tensor("bj_sb", [128, VOCAB], f32))
        pc_sb = e(nc.sbuf_tensor("pc_sb", [128, 12], f32))
        ET_sb = e(nc.sbuf_tensor("ET_sb", [128, HT, M], f32))
        DT_sb = e(nc.sbuf_tensor("DT_sb", [128, HT, B * U], f32))
        # polynomial temps, one m-pair each ([128, 2, 512])
        S_sb = e(nc.sbuf_tensor("S_sb", [128, 2, 512], f32))
        xc_sb = e(nc.sbuf_tensor("xc_sb", [128, 2, 512], f32))
        y_sb = e(nc.sbuf_tensor("y_sb", [128, 2, 512], f32))
        p_sb = e(nc.sbuf_tensor("p_sb", [128, 2, 512], f32))
        q_sb = e(nc.sbuf_tensor("q_sb", [128, 2, 512], f32))
        jt_r = e(nc.sbuf_tensor("jt_r", [128, NJT, 2, HT * 128], f32r))
        ot_sb = e(nc.sbuf_tensor("ot_sb", [128, NOT, VOCAB], f32))
        ps = [
            e(nc.psum_tensor(f"ps{i}", [128, VOCAB], f32)) for i in range(NPS)
        ]
        s_in = e(nc.semaphore("s_in"))
        s_cp = e(nc.semaphore("s_cp"))
        s_pe = e(nc.semaphore("s_pe"))
        s_dve = e(nc.semaphore("s_dve"))
        s_outd = e(nc.semaphore("s_outd"))
        block = e(nc.Block())

        @block.sync
        def _(sync):
            for sb, dr in (
                (encT_sb, encT),
                (decT_sb, decT),
                (WencT_sb, WencT),
                (WdecT_sb, WdecT),
                (WjT_stage, WjT),
                (bsum_sb, bsum),
                (bj_sb, bj),
                (pc_sb, pc),
            ):
                sync.dma_start(out=sb[:], in_=dr[:]).then_inc(s_in, 16)
            for rep in range(reps):
                for m in range(M):
                    c = rep * M + m
                    sync.wait_ge(s_dve, 9 + c + 1)
                    sync.dma_start(out=out[m], in_=ot_sb[:, c % NOT, :]).then_inc(
                        s_outd, 16
                    )
            sync.wait_ge(s_outd, 16 * M * reps)
            if timing:
                sync.dma_start(out=tok[:], in_=bsum_sb[:]).then_inc(s_in, 16)
                sync.wait_ge(s_in, 16 * (N_IN_DMA + 1))

        @block.tensor
        def _(pe):
            pe.wait_ge(s_in, 16 * N_IN_DMA)
            # E^T: ps[hi][:, 0:M] (bank 2*hi)
            for hi in range(HT):
                for ki in range(KT):
                    mm = pe.matmul(
                        ps[hi][:, 0:M],
                        WencT_sb[:, ki, hi * 128 : (hi + 1) * 128],
                        encT_sb[:, ki, :],
                        start=(ki == 0),
                        stop=(ki == KT - 1),
                    )
                mm.then_inc(s_pe, 1)
            # D^T: ps[hi][:, 512:1024] (bank 2*hi+1)
            for hi in range(HT):
                for ki in range(KT):
                    mm = pe.matmul(
                        ps[hi][:, 512 : 512 + B * U],
                        WdecT_sb[:, ki, hi * 128 : (hi + 1) * 128],
                        decT_sb[:, ki, :],
                        start=(ki == 0),
                        stop=(ki == KT - 1),
                    )
                mm.then_inc(s_pe, 1)
            # main loop: one s_cp wait per pair.  s_cp >= g+1 also implies
            # (a) setup drains done (DVE emits pair 0 after setup) and
            # (b) psum slot m%NPS drained (DVE drains pair g-2 before
            # producing pair g, and PE<=2g+1 needs drains<=2g-3 only).
            for rep in range(reps):
                for g in range(NPAIR):
                    gg = rep * NPAIR + g
                    pe.wait_ge(s_cp, gg + 1)
                    for half in range(2):
                        c = 2 * gg + half
                        for hi in range(HT):
                            for vi in range(2):
                                mm = pe.matmul(
                                    ps[c % NPS][:, vi * 512 : (vi + 1) * 512],
                                    jt_r[
                                        :,
                                        gg % NJT,
                                        half,
                                        hi * 128 : (hi + 1) * 128,
                                    ],
                                    WjT_sb[:, hi, vi * 512 : (vi + 1) * 512],
                                    start=(hi == 0),
                                    stop=(hi == HT - 1),
                                )
                        mm.then_inc(s_pe, 1)

        @block.vector
        def _(dve):
            dve.wait_ge(s_in, 16 * N_IN_DMA)
            for hi in range(HT):
                dve.wait_ge(s_pe, hi + 1)
                dve.tensor_copy(ET_sb[:, hi, :], ps[hi][:, 0:M]).then_inc(s_dve, 1)
            for hi in range(HT):
                dve.wait_ge(s_pe, 4 + hi + 1)
                dve.tensor_scalar_add(
                    DT_sb[:, hi, :],
                    ps[hi][:, 512 : 512 + B * U],
                    bsum_sb[:, hi : hi + 1],
                ).then_inc(s_dve, 1)
            dve.tensor_copy(WjT_sb[:], WjT_stage[:]).then_inc(s_dve, 1)
            # s_dve = 9 after setup

            def drain(c):
                dve.wait_ge(s_pe, 8 + c + 1)
                if c >= NOT:
                    dve.wait_ge(s_outd, 16 * ((c - NOT) + 1))  # ot slot free
                dve.tensor_tensor(
                    ot_sb[:, c % NOT, :],
                    ps[c % NPS][:, :],
                    bj_sb[:, :],
                    Add,
                ).then_inc(s_dve, 1)

            # polynomial constants as [128,1] AP slices of pc_sb:
            # pc[:, k] = POLY_C[k] (k=0..6), pc[:, 7] = CLAMP, pc[:, 8] = -CLAMP
            NG = reps * NPAIR
            for gg in range(NG):
                m0 = (2 * gg) % M
                b = m0 // TC
                # drains first: frees psum slots two pairs back before
                # this pair's jt is posted
                if gg >= 2:
                    drain(2 * (gg - 2))
                    drain(2 * (gg - 2) + 1)
                # S = DT + e_m
                for half in range(2):
                    m = m0 + half
                    for hi in range(HT):
                        dve.tensor_scalar_add(
                            S_sb[:, half, hi * 128 : (hi + 1) * 128],
                            DT_sb[:, hi, b * 128 : (b + 1) * 128],
                            ET_sb[:, hi, m : m + 1],
                        )
                # xc = clamp(S, +-CLAMP); y = xc*xc
                dve.tensor_scalar_min(q_sb[:], S_sb[:], pc_sb[:, 7:8])
                dve.tensor_scalar_max(xc_sb[:], q_sb[:], pc_sb[:, 8:9])
                dve.tensor_tensor(y_sb[:], xc_sb[:], xc_sb[:], Mult)
                # Horner: p = C6*y + C5; then p = p*y + Ck
                dve.tensor_scalar_mul(q_sb[:], y_sb[:], pc_sb[:, 6:7])
                dve.tensor_scalar_add(p_sb[:], q_sb[:], pc_sb[:, 5:6])
                for k in (4, 3, 2, 1, 0):
                    dve.tensor_tensor(q_sb[:], p_sb[:], y_sb[:], Mult)
                    dve.tensor_scalar_add(p_sb[:], q_sb[:], pc_sb[:, k : k + 1])
                # jt = p*xc (f32, staged in S_sb), then re-round to f32r
                dve.tensor_tensor(S_sb[:], p_sb[:], xc_sb[:], Mult)
                # jt_r slot free once PE consumed pair gg-NJT
                if gg >= NJT:
                    dve.wait_ge(s_pe, 8 + 2 * (gg - NJT) + 2)
                dve.tensor_copy(
                    jt_r[:, gg % NJT, :, :], S_sb[:]
                ).then_inc(s_cp, 1)
            # tail drains
            for c in range(2 * (NG - 2), 2 * NG):
                drain(c)

    return nc


def _tile_k(a):
    """[K, X] -> [128, K//128, X] with k = kt*128 + p."""
    k, x = a.shape
    return np.ascontiguousarray(a.reshape(k // 128, 128, x).transpose(1, 0, 2))


def _prep_inputs(enc_out, dec_out, W_enc, b_enc, W_dec, b_dec, W_joint, b_joint):
    enc_out = np.asarray(enc_out, dtype=np.float32)
    dec_out = np.asarray(dec_out, dtype=np.float32)
    common = {
        "decT": _tile_k(np.ascontiguousarray(dec_out.reshape(B * U, DEC_DIM).T)),
        "WencT": _tile_k(np.ascontiguousarray(np.asarray(W_enc, np.float32).T)),
        "WdecT": _tile_k(np.ascontiguousarray(np.asarray(W_dec, np.float32).T)),
        "WjT": _tile_k(np.ascontiguousarray(np.asarray(W_joint, np.float32).T)),
        "bsum": np.ascontiguousarray(
            (np.asarray(b_enc, np.float32) + np.asarray(b_dec, np.float32))
            .reshape(HT, 128)
            .T
        ),
        "bj": np.ascontiguousarray(
            np.broadcast_to(np.asarray(b_joint, np.float32), (128, VOCAB))
        ),
        "pc": np.ascontiguousarray(
            np.broadcast_to(
                np.array(
                    POLY_C + [CLAMP, -CLAMP, 0.0, 0.0, 0.0], np.float32
                ),
                (128, 12),
            )
        ),
    }
    in_maps = []
    for i in range(NCORES):
        sl = enc_out[:, i * TC : (i + 1) * TC, :].reshape(M, ENC_DIM)
        m = dict(common)
        m["encT"] = _tile_k(np.ascontiguousarray(sl.T))
        in_maps.append(m)
    return in_maps


def run(in_maps, trace=False, **kw):
    from concourse.bass_utils import run_bass_kernel_spmd

    if "nc" not in _CACHE:
        _CACHE["nc"] = _build_bass()
    return run_bass_kernel_spmd(
        _CACHE["nc"], in_maps, list(range(NCORES)), trace=trace, **kw
    )


def time_kernel(in_maps, reps_list=(1, 9), n_meas=3):
    """HW time per main-loop pass via rep-count wall-clock deltas.

    Timing variants write to internal DRAM (tiny external output), so the
    axon transfer cost is identical across rep counts and cancels in the
    delta.
    """
    import time
    from concourse.bass_utils import run_bass_kernel_spmd

    walls = {}
    for reps in reps_list:
        key = f"t{reps}"
        if key not in _CACHE:
            _CACHE[key] = _build_bass(reps=reps, timing=True)
        nc = _CACHE[key]
        run_bass_kernel_spmd(nc, in_maps, list(range(NCORES)))  # compile+warm
        ts = []
        for _ in range(n_meas):
            t0 = time.time()
            run_bass_kernel_spmd(nc, in_maps, list(range(NCORES)))
            ts.append(time.time() - t0)
        walls[reps] = min(ts)
    r0, r1 = reps_list
    per_pass = (walls[r1] - walls[r0]) / (r1 - r0)
    return per_pass, walls


def kernel(enc_out, dec_out, W_enc, b_enc, W_dec, b_dec, W_joint, b_joint):
    import sys

    if "/opt/trn_rl_repo" not in sys.path:
        sys.path.insert(0, "/opt/trn_rl_repo")

    in_maps = _prep_inputs(
        enc_out, dec_out, W_enc, b_enc, W_dec, b_dec, W_joint, b_joint
    )
    res = run(in_maps)
    parts = [r["out"].reshape(B, TC, U, VOCAB) for r in res.results]
    return np.concatenate(parts, axis=1)


# revision 31
# speedup vs baseline: 2.9238x; 2.0752x over previous
"""RNN-T JointNetwork kernel for 8 Trainium2 NeuronCores (raw bass).

reference:
  e = enc @ W_enc.T + b_enc          # [B,T,H]
  d = dec @ W_dec.T + b_dec          # [B,U,H]
  j = tanh(e[:,:,None,:] + d[:,None,:,:])
  out = j @ W_joint.T + b_joint      # [B,T,U,V]

Sharding: T (256) split 8 ways -> 32 t-rows per core; host concatenates
along T.

This platform charges a large, roughly flat cost per engine instruction
and a multi-ms latency for cross-engine dependencies that actually
block.  The kernel therefore (a) minimizes instruction count per
engine, (b) keeps ACT on its fast f32-output path (f32r ACT output is
~4x slower per instruction here), and (c) keeps the jt producer chain
(ACT tanh -> DVE f32r re-round) faster per pair than PE so PE's waits
are pre-satisfied.  DVE emits the psum drains *before* posting each
pair so PE's psum slots are implied free by its single s_cp wait; SP
just streams the output DMAs.

Per-core dataflow:
  PE:   E^T[h,m], D^T[h,n] projections (f32, setup); per m-pair the
        vocab matmul psum[u,v] += jt[h,u]^T WjT^T[h,v] (f32r, 4 k-tiles
        x 2 v-banks per m), one semaphore wait per pair
  ACT:  jt_sb[h,u] = tanh(DT + e_m) via the per-partition bias port,
        f32 out, running up to NJTM=16 m ahead
  DVE:  setup drains (ET/DT + bias fold, WjT f32->f32r cast); per pair:
        2 psum drains (+b_joint), 1 f32->f32r re-round copy to jt_r
  SP:   all DMA (inputs once, one 512KB contiguous output row per m)
"""

import numpy as np

B, T, U = 4, 256, 128
ENC_DIM = DEC_DIM = HID = 512
VOCAB = 1024
NCORES = 8
TC = T // NCORES        # 32 t-rows per core
M = B * TC              # 128 (b,t) rows per core
KT = HID // 128         # 4 contraction tiles
HT = HID // 128         # 4 h tiles
NJT = 8                 # jt_r pair-ring depth (f32r, DVE->PE)
NJTM = 16               # jt_sb m-ring depth (f32, ACT->DVE)
NOT = 6                 # output staging buffers
NPS = 4                 # psum tiles in flight

# tanh(x) ~ xc*(C[0] + C[1] y + ... + C[6] y^6), y = xc^2, xc clamped
CLAMP = 3.75
POLY_C = [
    9.868656054e-01,
    -2.815523407e-01,
    6.868982108e-02,
    -1.060985507e-02,
    9.525619066e-04,
    -4.506074475e-05,
    8.650786272e-07,
]

_CACHE = {}


def _build_bass(reps=1, timing=False):
    import concourse.bass as bass
    import concourse.mybir as mybir

    f32 = mybir.dt.float32
    f32r = mybir.dt.float32r
    Add = mybir.AluOpType.add
    Tanh = mybir.ActivationFunctionType.Tanh

    nc = bass.Bass()
    encT = nc.declare_dram_parameter("encT", [128, KT, M], f32, isOutput=False)
    decT = nc.declare_dram_parameter("decT", [128, KT, B * U], f32, isOutput=False)
    WencT = nc.declare_dram_parameter("WencT", [128, KT, HID], f32, isOutput=False)
    WdecT = nc.declare_dram_parameter("WdecT", [128, KT, HID], f32, isOutput=False)
    WjT = nc.declare_dram_parameter("WjT", [128, HT, VOCAB], f32, isOutput=False)
    bsum = nc.declare_dram_parameter("bsum", [128, HT], f32, isOutput=False)
    bj = nc.declare_dram_parameter("bj", [128, VOCAB], f32, isOutput=False)
    pc = nc.declare_dram_parameter("pc", [128, 12], f32, isOutput=False)
    if timing:
        out = nc.dram_tensor("out_i", [M, U, VOCAB], f32)
        tok = nc.declare_dram_parameter("tok", [128, 4], f32, isOutput=True)
    else:
        out = nc.declare_dram_parameter("out", [M, U, VOCAB], f32, isOutput=True)

    N_IN_DMA = 8
    NPAIR = M // 2  # 64 pairs per pass

    from contextlib import ExitStack

    with ExitStack() as ctx:
        e = ctx.enter_context
        encT_sb = e(nc.sbuf_tensor("encT_sb", [128, KT, M], f32))
        decT_sb = e(nc.sbuf_tensor("decT_sb", [128, KT, B * U], f32))
        WencT_sb = e(nc.sbuf_tensor("WencT_sb", [128, KT, HID], f32))
        WdecT_sb = e(nc.sbuf_tensor("WdecT_sb", [128, KT, HID], f32))
        WjT_stage = e(nc.sbuf_tensor("WjT_stage", [128, HT, VOCAB], f32))
        WjT_sb = e(nc.sbuf_tensor("WjT_sb", [128, HT, VOCAB], f32r))
        bsum_sb = e(nc.sbuf_tensor("bsum_sb", [128, HT], f32))
        bj_sb = e(nc.sbuf_tensor("bj_sb", [128, VOCAB], f32))
        pc_sb = e(nc.sbuf_tensor("pc_sb", [128, 12], f32))
        ET_sb = e(nc.sbuf_tensor("ET_sb", [128, HT, M], f32))
        DT_sb = e(nc.sbuf_tensor("DT_sb", [128, HT, B * U], f32))
        # ACT writes tanh as f32 (fast path) into jt_sb; DVE re-rounds
        # each pair to f32r in jt_r for the full-rate PE matmul.
        jt_sb = e(nc.sbuf_tensor("jt_sb", [128, NJTM, HT, 128], f32))
        jt_r = e(nc.sbuf_tensor("jt_r", [128, NJT, 2, HT, 128], f32r))
        ot_sb = e(nc.sbuf_tensor("ot_sb", [128, NOT, VOCAB], f32))
        ps = [
            e(nc.psum_tensor(f"ps{i}", [128, VOCAB], f32)) for i in range(NPS)
        ]
        s_in = e(nc.semaphore("s_in"))
        s_act = e(nc.semaphore("s_act"))
        s_cp = e(nc.semaphore("s_cp"))
        s_pe = e(nc.semaphore("s_pe"))
        s_dve = e(nc.semaphore("s_dve"))
        s_outd = e(nc.semaphore("s_outd"))
        block = e(nc.Block())

        @block.sync
        def _(sync):
            for sb, dr in (
                (encT_sb, encT),
                (decT_sb, decT),
                (WencT_sb, WencT),
                (WdecT_sb, WdecT),
                (WjT_stage, WjT),
                (bsum_sb, bsum),
                (bj_sb, bj),
                (pc_sb, pc),
            ):
                sync.dma_start(out=sb[:], in_=dr[:]).then_inc(s_in, 16)
            for rep in range(reps):
                for m in range(M):
                    c = rep * M + m
                    sync.wait_ge(s_dve, 9 + c + 1)
                    sync.dma_start(out=out[m], in_=ot_sb[:, c % NOT, :]).then_inc(
                        s_outd, 16
                    )
            sync.wait_ge(s_outd, 16 * M * reps)
            if timing:
                sync.dma_start(out=tok[:], in_=bsum_sb[:]).then_inc(s_in, 16)
                sync.wait_ge(s_in, 16 * (N_IN_DMA + 1))

        @block.tensor
        def _(pe):
            pe.wait_ge(s_in, 16 * N_IN_DMA)
            # E^T: ps[hi][:, 0:M] (bank 2*hi)
            for hi in range(HT):
                for ki in range(KT):
                    mm = pe.matmul(
                        ps[hi][:, 0:M],
                        WencT_sb[:, ki, hi * 128 : (hi + 1) * 128],
                        encT_sb[:, ki, :],
                        start=(ki == 0),
                        stop=(ki == KT - 1),
                    )
                mm.then_inc(s_pe, 1)
            # D^T: ps[hi][:, 512:1024] (bank 2*hi+1)
            for hi in range(HT):
                for ki in range(KT):
                    mm = pe.matmul(
                        ps[hi][:, 512 : 512 + B * U],
                        WdecT_sb[:, ki, hi * 128 : (hi + 1) * 128],
                        decT_sb[:, ki, :],
                        start=(ki == 0),
                        stop=(ki == KT - 1),
                    )
                mm.then_inc(s_pe, 1)
            # main loop: one s_cp wait per pair.  s_cp >= g+1 also implies
            # (a) setup drains done (DVE emits pair 0 after setup) and
            # (b) psum slot m%NPS drained (DVE drains pair g-2 before
            # producing pair g, and PE<=2g+1 needs drains<=2g-3 only).
            for rep in range(reps):
                for g in range(NPAIR):
                    gg = rep * NPAIR + g
                    pe.wait_ge(s_cp, gg + 1)
                    for half in range(2):
                        c = 2 * gg + half
                        for hi in range(HT):
                            for vi in range(2):
                                mm = pe.matmul(
                                    ps[c % NPS][:, vi * 512 : (vi + 1) * 512],
                                    jt_r[:, gg % NJT, half, hi, :],
                                    WjT_sb[:, hi, vi * 512 : (vi + 1) * 512],
                                    start=(hi == 0),
                                    stop=(hi == HT - 1),
                                )
                        mm.then_inc(s_pe, 1)

        @block.scalar
        def _(act):
            act.wait_ge(s_dve, 8)  # ET/DT ready
            for rep in range(reps):
                for m in range(M):
                    c = rep * M + m
                    b = m // TC
                    # jt_sb slot c%NJTM free once DVE copied pair (c-NJTM)/2
                    if c >= NJTM and c % 2 == 0:
                        act.wait_ge(s_cp, (c - NJTM) // 2 + 1)
                    for hi in range(HT):
                        a = act.activation(
                            jt_sb[:, c % NJTM, hi, :],
                            DT_sb[:, hi, b * 128 : (b + 1) * 128],
                            Tanh,
                            bias=ET_sb[:, hi, m : m + 1],
                        )
                    a.then_inc(s_act, 1)

        @block.vector
        def _(dve):
            dve.wait_ge(s_in, 16 * N_IN_DMA)
            for hi in range(HT):
                dve.wait_ge(s_pe, hi + 1)
                dve.tensor_copy(ET_sb[:, hi, :], ps[hi][:, 0:M]).then_inc(s_dve, 1)
            for hi in range(HT):
                dve.wait_ge(s_pe, 4 + hi + 1)
                dve.tensor_scalar_add(
                    DT_sb[:, hi, :],
                    ps[hi][:, 512 : 512 + B * U],
                    bsum_sb[:, hi : hi + 1],
                ).then_inc(s_dve, 1)
            dve.tensor_copy(WjT_sb[:], WjT_stage[:]).then_inc(s_dve, 1)
            # s_dve = 9 after setup

            def drain(c, waits=True):
                if waits:
                    dve.wait_ge(s_pe, 8 + c + 1)
                    if c >= NOT:
                        dve.wait_ge(s_outd, 16 * ((c - NOT) + 1))  # ot slot
                dve.tensor_tensor(
                    ot_sb[:, c % NOT, :],
                    ps[c % NPS][:, :],
                    bj_sb[:, :],
                    Add,
                ).then_inc(s_dve, 1)

            def drain_pair(c0):
                # one s_pe wait + one s_outd wait covering both drains
                dve.wait_ge(s_pe, 8 + c0 + 2)
                if c0 + 1 >= NOT:
                    dve.wait_ge(s_outd, 16 * ((c0 + 1 - NOT) + 1))
                drain(c0, waits=False)
                drain(c0 + 1, waits=False)

            NG = reps * NPAIR
            for gg in range(NG):
                # drains first: frees psum slots two pairs back before
                # this pair's jt is posted
                if gg >= 2:
                    drain_pair(2 * (gg - 2))
                # re-round ACT's pair to f32r for PE
                dve.wait_ge(s_act, 2 * gg + 2)  # both m of pair tanh'd
                # jt_r slot free once PE consumed pair gg-NJT
                if gg >= NJT:
                    dve.wait_ge(s_pe, 8 + 2 * (gg - NJT) + 2)
                m0 = (2 * gg) % NJTM
                dve.tensor_copy(
                    jt_r[:, gg % NJT, :, :, :],
                    jt_sb[:, m0 : m0 + 2, :, :],
                ).then_inc(s_cp, 1)
            # tail drains
            for c in range(2 * (NG - 2), 2 * NG):
                drain(c)

    return nc


def _tile_k(a):
    """[K, X] -> [128, K//128, X] with k = kt*128 + p."""
    k, x = a.shape
    return np.ascontiguousarray(a.reshape(k // 128, 128, x).transpose(1, 0, 2))


def _prep_inputs(enc_out, dec_out, W_enc, b_enc, W_dec, b_dec, W_joint, b_joint):
    enc_out = np.asarray(enc_out, dtype=np.float32)
    dec_out = np.asarray(dec_out, dtype=np.float32)
    common = {
        "decT": _tile_k(np.ascontiguousarray(dec_out.reshape(B * U, DEC_DIM).T)),
        "WencT": _tile_k(np.ascontiguousarray(np.asarray(W_enc, np.float32).T)),
        "WdecT": _tile_k(np.ascontiguousarray(np.asarray(W_dec, np.float32).T)),
        "WjT": _tile_k(np.ascontiguousarray(np.asarray(W_joint, np.float32).T)),
        "bsum": np.ascontiguousarray(
            (np.asarray(b_enc, np.float32) + np.asarray(b_dec, np.float32))
            .reshape(HT, 128)
            .T
        ),
        "bj": np.ascontiguousarray(
            np.broadcast_to(np.asarray(b_joint, np.float32), (128, VOCAB))
        ),
        "pc": np.ascontiguousarray(
            np.broadcast_to(
                np.array(
                    POLY_C + [CLAMP, -CLAMP, 0.0, 0.0, 0.0], np.float32
                ),
                (128, 12),
            )
        ),
    }
    in_maps = []
    for i in range(NCORES):
        sl = enc_out[:, i * TC : (i + 1) * TC, :].reshape(M, ENC_DIM)
        m = dict(common)
        m["encT"] = _tile_k(np.ascontiguousarray(sl.T))
        in_maps.append(m)
    return in_maps


def run(in_maps, trace=False, **kw):
    from concourse.bass_utils import run_bass_kernel_spmd

    if "nc" not in _CACHE:
        _CACHE["nc"] = _build_bass()
    return run_bass_kernel_spmd(
        _CACHE["nc"], in_maps, list(range(NCORES)), trace=trace, **kw
    )


def time_kernel(in_maps, reps_list=(1, 9), n_meas=3):
    """HW time per main-loop pass via rep-count wall-clock deltas.

    Timing variants write to internal DRAM (tiny external output), so the
    axon transfer cost is identical across rep counts and cancels in the
    delta.
    """
    import time
    from concourse.bass_utils import run_bass_kernel_spmd

    walls = {}
    for reps in reps_list:
        key = f"t{reps}"
        if key not in _CACHE:
            _CACHE[key] = _build_bass(reps=reps, timing=True)
        nc = _CACHE[key]
        run_bass_kernel_spmd(nc, in_maps, list(range(NCORES)))  # compile+warm
        ts = []
        for _ in range(n_meas):
            t0 = time.time()
            run_bass_kernel_spmd(nc, in_maps, list(range(NCORES)))
            ts.append(time.time() - t0)
        walls[reps] = min(ts)
    r0, r1 = reps_list
    per_pass = (walls[r1] - walls[r0]) / (r1 - r0)
    return per_pass, walls


def kernel(enc_out, dec_out, W_enc, b_enc, W_dec, b_dec, W_joint, b_joint):
    import sys

    if "/opt/trn_rl_repo" not in sys.path:
        sys.path.insert(0, "/opt/trn_rl_repo")

    in_maps = _prep_inputs(
        enc_out, dec_out, W_enc, b_enc, W_dec, b_dec, W_joint, b_joint
    )
    res = run(in_maps)
    parts = [r["out"].reshape(B, TC, U, VOCAB) for r in res.results]
    return np.concatenate(parts, axis=1)


# revision 41
# speedup vs baseline: 3.6581x; 1.2511x over previous
"""RNN-T JointNetwork kernel for 8 Trainium2 NeuronCores (raw bass).

reference:
  e = enc @ W_enc.T + b_enc          # [B,T,H]
  d = dec @ W_dec.T + b_dec          # [B,U,H]
  j = tanh(e[:,:,None,:] + d[:,None,:,:])
  out = j @ W_joint.T + b_joint      # [B,T,U,V]

Sharding: T (256) split 8 ways -> 32 t-rows per core; host concatenates
along T.

This platform charges a large, roughly flat cost per engine instruction
and a multi-ms latency for cross-engine dependencies that actually
block.  The kernel therefore (a) minimizes instruction count per
engine, (b) keeps ACT on its fast f32-output path (f32r ACT output is
~4x slower per instruction here), and (c) keeps the jt producer chain
(ACT tanh -> DVE f32r re-round) faster per pair than PE so PE's waits
are pre-satisfied.  DVE emits the psum drains *before* posting each
pair so PE's psum slots are implied free by its single s_cp wait; SP
just streams the output DMAs.

Per-core dataflow:
  PE:   E^T[h,m], D^T[h,n] projections (f32, setup); per m-pair the
        vocab matmul psum[u,v] += jt[h,u]^T WjT^T[h,v] (f32r, 4 k-tiles
        x 2 v-banks per m), one semaphore wait per pair
  ACT:  jt_sb[h,u] = tanh(DT + e_m) via the per-partition bias port,
        f32 out, running up to NJTM=16 m ahead
  DVE:  setup drains (ET/DT + bias fold, WjT f32->f32r cast); per pair:
        2 psum drains (+b_joint), 1 f32->f32r re-round copy to jt_r
  SP:   all DMA (inputs once, one 512KB contiguous output row per m)
"""

import numpy as np

B, T, U = 4, 256, 128
ENC_DIM = DEC_DIM = HID = 512
VOCAB = 1024
NCORES = 8
TC = T // NCORES        # 32 t-rows per core
M = B * TC              # 128 (b,t) rows per core
KT = HID // 128         # 4 contraction tiles
HT = HID // 128         # 4 h tiles
NJT = 8                 # jt_r pair-ring depth (f32r, DVE->PE)
NJTM = 16               # jt_sb m-ring depth (f32, ACT->DVE)
NOT = 6                 # output staging buffers
NPS = 4                 # psum tiles in flight

# tanh(x) ~ xc*(C[0] + C[1] y + ... + C[6] y^6), y = xc^2, xc clamped
CLAMP = 3.75
POLY_C = [
    9.868656054e-01,
    -2.815523407e-01,
    6.868982108e-02,
    -1.060985507e-02,
    9.525619066e-04,
    -4.506074475e-05,
    8.650786272e-07,
]

_CACHE = {}


def _build_bass(reps=1, timing=False):
    import concourse.bass as bass
    import concourse.mybir as mybir

    f32 = mybir.dt.float32
    f32r = mybir.dt.float32r
    Add = mybir.AluOpType.add
    Tanh = mybir.ActivationFunctionType.Tanh

    nc = bass.Bass()
    encT = nc.declare_dram_parameter("encT", [128, KT, M], f32, isOutput=False)
    decT = nc.declare_dram_parameter("decT", [128, KT, B * U], f32, isOutput=False)
    WencT = nc.declare_dram_parameter("WencT", [128, KT, HID], f32, isOutput=False)
    WdecT = nc.declare_dram_parameter("WdecT", [128, KT, HID], f32, isOutput=False)
    WjT = nc.declare_dram_parameter("WjT", [128, HT, VOCAB], f32, isOutput=False)
    bsum = nc.declare_dram_parameter("bsum", [128, HT], f32, isOutput=False)
    bj = nc.declare_dram_parameter("bj", [128, VOCAB], f32, isOutput=False)
    pc = nc.declare_dram_parameter("pc", [128, 12], f32, isOutput=False)
    if timing:
        out = nc.dram_tensor("out_i", [M, U, VOCAB], f32)
        tok = nc.declare_dram_parameter("tok", [128, 4], f32, isOutput=True)
    else:
        out = nc.declare_dram_parameter("out", [M, U, VOCAB], f32, isOutput=True)

    N_IN_DMA = 8
    NPAIR = M // 2  # 64 pairs per pass

    from contextlib import ExitStack

    with ExitStack() as ctx:
        e = ctx.enter_context
        encT_sb = e(nc.sbuf_tensor("encT_sb", [128, KT, M], f32))
        decT_sb = e(nc.sbuf_tensor("decT_sb", [128, KT, B * U], f32))
        WencT_sb = e(nc.sbuf_tensor("WencT_sb", [128, KT, HID], f32))
        WdecT_sb = e(nc.sbuf_tensor("WdecT_sb", [128, KT, HID], f32))
        WjT_stage = e(nc.sbuf_tensor("WjT_stage", [128, HT, VOCAB], f32))
        WjT_sb = e(nc.sbuf_tensor("WjT_sb", [128, HT, VOCAB], f32r))
        bsum_sb = e(nc.sbuf_tensor("bsum_sb", [128, HT], f32))
        bj_sb = e(nc.sbuf_tensor("bj_sb", [128, VOCAB], f32))
        pc_sb = e(nc.sbuf_tensor("pc_sb", [128, 12], f32))
        ET_sb = e(nc.sbuf_tensor("ET_sb", [128, HT, M], f32))
        DT_sb = e(nc.sbuf_tensor("DT_sb", [128, HT, B * U], f32))
        # ACT writes tanh as f32 (fast path) into jt_sb; DVE re-rounds
        # each pair to f32r in jt_r for the full-rate PE matmul.
        jt_sb = e(nc.sbuf_tensor("jt_sb", [128, NJTM, HT, 128], f32))
        jt_r = e(nc.sbuf_tensor("jt_r", [128, NJT, 2, HT, 128], f32r))
        ot_sb = e(nc.sbuf_tensor("ot_sb", [128, NOT, VOCAB], f32))
        ps = [
            e(nc.psum_tensor(f"ps{i}", [128, VOCAB], f32)) for i in range(NPS)
        ]
        s_in = e(nc.semaphore("s_in"))
        s_act = e(nc.semaphore("s_act"))
        s_cp = e(nc.semaphore("s_cp"))
        s_pe = e(nc.semaphore("s_pe"))
        s_dve = e(nc.semaphore("s_dve"))
        s_outd = e(nc.semaphore("s_outd"))
        block = e(nc.Block())

        @block.sync
        def _(sync):
            for sb, dr in (
                (encT_sb, encT),
                (decT_sb, decT),
                (WencT_sb, WencT),
                (WdecT_sb, WdecT),
                (WjT_stage, WjT),
                (bsum_sb, bsum),
                (bj_sb, bj),
                (pc_sb, pc),
            ):
                sync.dma_start(out=sb[:], in_=dr[:]).then_inc(s_in, 16)
            for rep in range(reps):
                for g in range(NPAIR):
                    gg = rep * NPAIR + g
                    sync.wait_ge(s_dve, 9 + 2 * gg + 2)  # both drains done
                    for half in range(2):
                        c = 2 * gg + half
                        sync.dma_start(
                            out=out[c % M], in_=ot_sb[:, c % NOT, :]
                        ).then_inc(s_outd, 16)
            sync.wait_ge(s_outd, 16 * M * reps)
            if timing:
                sync.dma_start(out=tok[:], in_=bsum_sb[:]).then_inc(s_in, 16)
                sync.wait_ge(s_in, 16 * (N_IN_DMA + 1))

        @block.tensor
        def _(pe):
            pe.wait_ge(s_in, 16 * N_IN_DMA)
            # E^T: ps[hi][:, 0:M] (bank 2*hi)
            for hi in range(HT):
                for ki in range(KT):
                    mm = pe.matmul(
                        ps[hi][:, 0:M],
                        WencT_sb[:, ki, hi * 128 : (hi + 1) * 128],
                        encT_sb[:, ki, :],
                        start=(ki == 0),
                        stop=(ki == KT - 1),
                    )
                mm.then_inc(s_pe, 1)
            # D^T: ps[hi][:, 512:1024] (bank 2*hi+1)
            for hi in range(HT):
                for ki in range(KT):
                    mm = pe.matmul(
                        ps[hi][:, 512 : 512 + B * U],
                        WdecT_sb[:, ki, hi * 128 : (hi + 1) * 128],
                        decT_sb[:, ki, :],
                        start=(ki == 0),
                        stop=(ki == KT - 1),
                    )
                mm.then_inc(s_pe, 1)
            # main loop: one s_cp wait per pair.  s_cp >= g+1 also implies
            # (a) setup drains done (DVE emits pair 0 after setup) and
            # (b) psum slot m%NPS drained (DVE drains pair g-2 before
            # producing pair g, and PE<=2g+1 needs drains<=2g-3 only).
            for rep in range(reps):
                for g in range(NPAIR):
                    gg = rep * NPAIR + g
                    pe.wait_ge(s_cp, gg + 1)
                    for half in range(2):
                        c = 2 * gg + half
                        for hi in range(HT):
                            for vi in range(2):
                                mm = pe.matmul(
                                    ps[c % NPS][:, vi * 512 : (vi + 1) * 512],
                                    jt_r[:, gg % NJT, half, hi, :],
                                    WjT_sb[:, hi, vi * 512 : (vi + 1) * 512],
                                    start=(hi == 0),
                                    stop=(hi == HT - 1),
                                )
                        mm.then_inc(s_pe, 1)

        @block.scalar
        def _(act):
            act.wait_ge(s_dve, 8)  # ET/DT ready
            for rep in range(reps):
                for m in range(M):
                    c = rep * M + m
                    b = m // TC
                    # jt_sb slot c%NJTM free once DVE copied pair (c-NJTM)/2
                    if c >= NJTM and c % 2 == 0:
                        act.wait_ge(s_cp, (c - NJTM) // 2 + 1)
                    for hi in range(HT):
                        a = act.activation(
                            jt_sb[:, c % NJTM, hi, :],
                            DT_sb[:, hi, b * 128 : (b + 1) * 128],
                            Tanh,
                            bias=ET_sb[:, hi, m : m + 1],
                        )
                    a.then_inc(s_act, 1)

        @block.vector
        def _(dve):
            dve.wait_ge(s_in, 16 * N_IN_DMA)
            for hi in range(HT):
                dve.wait_ge(s_pe, hi + 1)
                dve.tensor_copy(ET_sb[:, hi, :], ps[hi][:, 0:M]).then_inc(s_dve, 1)
            for hi in range(HT):
                dve.wait_ge(s_pe, 4 + hi + 1)
                dve.tensor_scalar_add(
                    DT_sb[:, hi, :],
                    ps[hi][:, 512 : 512 + B * U],
                    bsum_sb[:, hi : hi + 1],
                ).then_inc(s_dve, 1)
            dve.tensor_copy(WjT_sb[:], WjT_stage[:]).then_inc(s_dve, 1)
            # s_dve = 9 after setup

            def drain(c, waits=True):
                if waits:
                    dve.wait_ge(s_pe, 8 + c + 1)
                    if c >= NOT:
                        dve.wait_ge(s_outd, 16 * ((c - NOT) + 1))  # ot slot
                dve.tensor_tensor(
                    ot_sb[:, c % NOT, :],
                    ps[c % NPS][:, :],
                    bj_sb[:, :],
                    Add,
                ).then_inc(s_dve, 1)

            def drain_pair(c0):
                # one s_pe wait + one s_outd wait covering both drains
                dve.wait_ge(s_pe, 8 + c0 + 2)
                if c0 + 1 >= NOT:
                    dve.wait_ge(s_outd, 16 * ((c0 + 1 - NOT) + 1))
                drain(c0, waits=False)
                drain(c0 + 1, waits=False)

            NG = reps * NPAIR
            for gg in range(NG):
                # drains first: frees psum slots two pairs back before
                # this pair's jt is posted
                if gg >= 2:
                    drain_pair(2 * (gg - 2))
                # re-round ACT's pair to f32r for PE.  (The jt_r slot-free
                # condition s_pe >= 8+2(gg-NJT)+2 is implied by the
                # drain_pair wait above, which targets 8+2(gg-2)+2.)
                dve.wait_ge(s_act, 2 * gg + 2)  # both m of pair tanh'd
                m0 = (2 * gg) % NJTM
                dve.tensor_copy(
                    jt_r[:, gg % NJT, :, :, :],
                    jt_sb[:, m0 : m0 + 2, :, :],
                ).then_inc(s_cp, 1)
            # tail drains
            for c in range(2 * (NG - 2), 2 * NG):
                drain(c)

    return nc


def _tile_k(a):
    """[K, X] -> [128, K//128, X] with k = kt*128 + p."""
    k, x = a.shape
    return np.ascontiguousarray(a.reshape(k // 128, 128, x).transpose(1, 0, 2))


def _prep_inputs(enc_out, dec_out, W_enc, b_enc, W_dec, b_dec, W_joint, b_joint):
    enc_out = np.asarray(enc_out, dtype=np.float32)
    dec_out = np.asarray(dec_out, dtype=np.float32)
    common = {
        "decT": _tile_k(np.ascontiguousarray(dec_out.reshape(B * U, DEC_DIM).T)),
        "WencT": _tile_k(np.ascontiguousarray(np.asarray(W_enc, np.float32).T)),
        "WdecT": _tile_k(np.ascontiguousarray(np.asarray(W_dec, np.float32).T)),
        "WjT": _tile_k(np.ascontiguousarray(np.asarray(W_joint, np.float32).T)),
        "bsum": np.ascontiguousarray(
            (np.asarray(b_enc, np.float32) + np.asarray(b_dec, np.float32))
            .reshape(HT, 128)
            .T
        ),
        "bj": np.ascontiguousarray(
            np.broadcast_to(np.asarray(b_joint, np.float32), (128, VOCAB))
        ),
        "pc": np.ascontiguousarray(
            np.broadcast_to(
                np.array(
                    POLY_C + [CLAMP, -CLAMP, 0.0, 0.0, 0.0], np.float32
                ),
                (128, 12),
            )
        ),
    }
    in_maps = []
    for i in range(NCORES):
        sl = enc_out[:, i * TC : (i + 1) * TC, :].reshape(M, ENC_DIM)
        m = dict(common)
        m["encT"] = _tile_k(np.ascontiguousarray(sl.T))
        in_maps.append(m)
    return in_maps


def run(in_maps, trace=False, **kw):
    from concourse.bass_utils import run_bass_kernel_spmd

    if "nc" not in _CACHE:
        _CACHE["nc"] = _build_bass()
    return run_bass_kernel_spmd(
        _CACHE["nc"], in_maps, list(range(NCORES)), trace=trace, **kw
    )


def time_kernel(in_maps, reps_list=(1, 9), n_meas=3):
    """HW time per main-loop pass via rep-count wall-clock deltas.

    Timing variants write to internal DRAM (tiny external output), so the
    axon transfer cost is identical across rep counts and cancels in the
    delta.
    """
    import time
    from concourse.bass_utils import run_bass_kernel_spmd

    walls = {}
    for reps in reps_list:
        key = f"t{reps}"
        if key not in _CACHE:
            _CACHE[key] = _build_bass(reps=reps, timing=True)
        nc = _CACHE[key]
        run_bass_kernel_spmd(nc, in_maps, list(range(NCORES)))  # compile+warm
        ts = []
        for _ in range(n_meas):
            t0 = time.time()
            run_bass_kernel_spmd(nc, in_maps, list(range(NCORES)))
            ts.append(time.time() - t0)
        walls[reps] = min(ts)
    r0, r1 = reps_list
    per_pass = (walls[r1] - walls[r0]) / (r1 - r0)
    return per_pass, walls


def kernel(enc_out, dec_out, W_enc, b_enc, W_dec, b_dec, W_joint, b_joint):
    import sys

    if "/opt/trn_rl_repo" not in sys.path:
        sys.path.insert(0, "/opt/trn_rl_repo")

    in_maps = _prep_inputs(
        enc_out, dec_out, W_enc, b_enc, W_dec, b_dec, W_joint, b_joint
    )
    res = run(in_maps)
    parts = [r["out"].reshape(B, TC, U, VOCAB) for r in res.results]
    return np.concatenate(parts, axis=1)
